# revision 15
# baseline (speedup 1.0000x reference)
"""Grouped per-channel Linear + ReLU on 8 TRN2 NeuronCores.

Problem: out[b,c,e] = relu(sum_s x[b,s,c] * W[c,s,e] + bias[c,e])
  x: (256, 2048, 32) f32, W: (32, 2048, 2048) f32, bias: (32, 2048) f32
  out: (256, 32, 2048) f32

Sharding: expert/channel parallel - core i computes channels [4i, 4i+4).
Each core runs 4 independent GEMMs of (256x2048)@(2048x2048) with the
contraction dim S on SBUF partitions, in fp16 (values are O(1), so fp16
gives ~3.6e-4 rel l2 error at full matmul rate and half the fp32 HBM
traffic).

Per-core roofline: PE ~110.5us (512 N=512 fp16 matmuls at ~216ns warm),
DMA ~117us (42 MB at ~358 GB/s per-core HBM share). DMA is the binding
floor, so the schedule keeps HBM saturated end to end:

- Host pre-transposes to per-partition-contiguous layouts (8 KB DMA
  descriptors): xt[c, p, k*B+b] and w[c, p, k, e].
- W streams as 1 MB (2 k-tile) chunks alternating across BOTH HWDGE
  rings (SP/sync + ACT/scalar), with ~12 MB of SBUF lookahead to ride
  through HBM arbitration jitter. Channel 0 ramps with 0.5 MB chunks
  (first one split in E halves) so the PE starts ~7us in.
- x slabs for channels 1-3 prefetch on the SWDGE (gpsimd) ring one
  channel ahead, gated behind a mid-channel W chunk so they can't
  starve the live W stream; channel 0's x rides the HWDGE rings at t=0.
- Outputs for channels 0-2 leave on the SWDGE ring (keeps the HWDGE
  FIFOs pure-W); the last channel's outputs leave eagerly per 512-col
  subtile on sync/scalar to shorten the tail.
- Eviction: VectorE adds the partition-broadcast bias (freeing the
  PSUM bank), ScalarE applies ReLU + fp16 cast. ttmp bufs=8 so the
  adds never serialize behind the activations.
"""

import os
import sys

for _p in ("/opt/trn_rl_repo", "/root/.axon_site/_ro/trn_rl_repo"):
    if os.path.isdir(_p) and _p not in sys.path:
        sys.path.insert(0, _p)

import numpy as np

import concourse.bacc as bacc
import concourse.mybir as mybir
from concourse import tile
from concourse.bass_utils import run_bass_kernel_spmd
from concourse.tile_rust import add_dep_helper

B, S, C, E = 256, 2048, 32, 2048
NCORES = 8
CPC = C // NCORES          # channels per core = 4
P = 128
KT = S // P                # 16 k-tiles
NBT = B // P               # 2 batch tiles
FREE = 512                 # matmul moving free dim (one PSUM bank of f32)
NET = E // FREE            # 4 e-tiles
KC = 2                     # k-tiles per W DMA chunk (1 MB chunks)
# W chunk lookahead. Two NeuronCores share one 716 GB/s HBM stack, so a
# core with deep buffers hoards the pair's bandwidth and starves its
# partner (the harness grades the SLOWEST core). 6 MB bounds the
# divergence while still riding through multi-us arbitration dips.
WBUFS = 6
NWARM = 48                 # PE warmup matmuls (N=128) during the DMA head

_nc_cache = {}


def _build():
    f16 = mybir.dt.float16
    f32 = mybir.dt.float32
    nc = bacc.Bacc(None, target_bir_lowering=False)
    # xt[c, p, k*B + b] = x[b, k*P + p, c] : 8 KB contiguous per partition
    xt = nc.dram_tensor("xt", [CPC, P, KT * B], f16, kind="ExternalInput")
    # w[c, p, k, e] = W[c, k*P + p, e] : k-tile rows contiguous per partition
    w = nc.dram_tensor("w", [CPC, P, KT, E], f16, kind="ExternalInput")
    bias = nc.dram_tensor("bias", [CPC, E], f32, kind="ExternalInput")
    out = nc.dram_tensor("out", [B, CPC, E], f16, kind="ExternalOutput")

    with tile.TileContext(nc) as tc:
        with (
            tc.tile_pool(name="const", bufs=1) as const,
            tc.tile_pool(name="xpool", bufs=2) as xpool,
            tc.tile_pool(name="bpool", bufs=2) as bpool,
            tc.tile_pool(name="bbpool", bufs=2) as bbpool,
            tc.tile_pool(name="ttmp", bufs=8) as ttmp,
            tc.tile_pool(name="wpool", bufs=WBUFS) as wpool,
            tc.tile_pool(name="opool", bufs=4) as opool,
            tc.tile_pool(name="psum", bufs=NBT * NET, space="PSUM") as psum,
        ):
            zbias = const.tile([P, 1], f32)
            nc.any.memset(zbias[:], 0.0)
            wz = const.tile([P, P], f16)
            nc.vector.memset(wz[:], 0.0)

            xtiles: dict[int, object] = {}
            btiles: dict[int, object] = {}

            def bias_broadcast(c):
                bsb = bpool.tile([1, E], f32, name="bsb")
                nc.gpsimd.dma_start(bsb[:], bias[c : c + 1, :])
                bbc = bbpool.tile([P, E], f32, name="bbc")
                nc.gpsimd.partition_broadcast(bbc[:], bsb[:])
                btiles[c] = bbc

            # Channel 0's x loads in 3 pieces: k0-1 races the first W chunk
            # on the scalar ring (sync carries pure W), the rest rides the
            # otherwise-idle SWDGE ring. The first matmuls only need piece A.
            xsb0 = xpool.tile([P, KT * B], f16, name="xsb")
            nc.sync.dma_start(xsb0[:, : 2 * B], xt[0, :, : 2 * B])
            nc.gpsimd.dma_start(xsb0[:, 2 * B : 8 * B], xt[0, :, 2 * B : 8 * B])
            nc.gpsimd.dma_start(xsb0[:, 8 * B :], xt[0, :, 8 * B :])
            xtiles[0] = xsb0



            def prefetch_channel(c, after):
                # next channel's x slab + bias on the SWDGE ring, held back
                # until mid-channel so it doesn't steal HBM bandwidth from
                # the live W stream (GpSimd is in-order: gating the slab
                # gates everything behind it too)
                xsb = xpool.tile([P, KT * B], f16, name="xsb")
                xdma = nc.gpsimd.dma_start(xsb[:], xt[c, :, :])
                add_dep_helper(
                    xdma.ins,
                    after.ins,
                    reason="x prefetch waits for mid-channel W chunk",
                )
                xtiles[c] = xsb
                bias_broadcast(c)

            qtog = [0]

            def weng():
                qtog[0] ^= 1
                return nc.sync if qtog[0] else nc.scalar

            for c in range(CPC):
                xsb = xtiles[c]
                ps = [
                    [
                        psum.tile([P, FREE], f32, name="ps")
                        for _ in range(NET)
                    ]
                    for _ in range(NBT)
                ]
                if c == 0:
                    # PE warmup: ~3.6us of zero matmuls into the first PSUM
                    # bank while the first x/W DMAs are in flight, so the
                    # HAM clock gate reaches 8/8 before the real matmuls
                    # start (otherwise the first ~35 run at 1.2 GHz and any
                    # ramp-delivery gap restarts the warmup window). The
                    # real k0 matmul (start=True) overwrites the garbage.
                    for _ in range(NWARM):
                        nc.tensor.matmul(
                            ps[0][0][:, :P], wz[:], wz[:], start=True, stop=True
                        )
                # W chunk schedule: channel 0 ramps with 1-k-tile chunks for
                # k0-k7 so, with strict per-ring FIFO ordering and queue
                # alternation, W k-tiles arrive in need order at most one
                # tile ahead; later channels stream 2-k-tile chunks.
                chunk_kts = [1] * 8 + [KC] * 4 if c == 0 else [KC] * (KT // KC)
                k = 0
                prefetched = False
                for ci, ckt in enumerate(chunk_kts):
                    wsb = wpool.tile([P, KC, E], f16, name="wsb")
                    if c == 0 and ci < 6:
                        # ramp k0-k5: E halves split across BOTH HWDGE
                        # rings (alternating which ring gets each half) so
                        # k-tiles complete in need order even when the two
                        # rings' contended rates differ
                        ea = nc.sync if ci % 2 == 0 else nc.scalar
                        eb = nc.scalar if ci % 2 == 0 else nc.sync
                        ea.dma_start(
                            wsb[:, :1, : E // 2], w[c, :, k : k + 1, : E // 2]
                        )
                        wdma = eb.dma_start(
                            wsb[:, :1, E // 2 :], w[c, :, k : k + 1, E // 2 :]
                        )
                    elif c == 0 and ci < 8:
                        # ramp k6-k7 ride the SWDGE ring: a third parallel
                        # delivery channel during the bandwidth-starved head
                        wdma = nc.gpsimd.dma_start(
                            wsb[:, :1, :], w[c, :, k : k + 1, :]
                        )
                        if ci == 7:
                            bias_broadcast(0)
                    else:
                        eng = weng()
                        wdma = eng.dma_start(
                            wsb[:, :ckt, :], w[c, :, k : k + ckt, :]
                        )
                    for kk in range(ckt):
                        for bt in range(NBT):
                            lhsT = xsb[:, k * B + bt * P : k * B + (bt + 1) * P]
                            for et in range(NET):
                                nc.tensor.matmul(
                                    ps[bt][et][:],
                                    lhsT,
                                    wsb[:, kk, et * FREE : (et + 1) * FREE],
                                    start=(k == 0),
                                    stop=(k == KT - 1),
                                )
                        k += 1
                    # gate at k>=8 on channel 0 so the slab queues behind the
                    # SWDGE ramp chunks (k6/k7) rather than blocking them
                    if (
                        not prefetched
                        and k >= (8 if c == 0 else 6)
                        and c + 1 < CPC
                    ):
                        prefetch_channel(c + 1, after=wdma)
                        prefetched = True
                # Evict: VectorE adds the broadcast bias (freeing the PSUM
                # bank), ScalarE applies ReLU + fp16 cast.
                bbc = btiles[c]
                last = c == CPC - 1
                oq = [0]
                for bt in range(NBT):
                    ot = opool.tile([P, E], f16)
                    for et in range(NET):
                        dst = ot[:, et * FREE : (et + 1) * FREE]
                        # fp16 tmp: halves ttmp SBUF; the pre-activation is
                        # O(1) so the fp16 rounding matches the final cast
                        tmp = ttmp.tile([P, FREE], f16, name="tmp")
                        nc.vector.tensor_add(
                            tmp[:],
                            ps[bt][et][:],
                            bbc[:, et * FREE : (et + 1) * FREE],
                        )
                        nc.scalar.activation(
                            dst,
                            tmp[:],
                            mybir.ActivationFunctionType.Relu,
                            bias=zbias[:],
                        )
                        if last:
                            # tail: eager per-subtile DMAs spread over all
                            # three rings (the W stream is finished by now);
                            # gpsimd takes the first two so its ~2us SWDGE
                            # setup cost overlaps the remaining evictions
                            oengs = [
                                nc.gpsimd, nc.gpsimd, nc.sync, nc.scalar,
                                nc.sync, nc.scalar, nc.sync, nc.scalar,
                            ]
                            oeng = oengs[oq[0]]
                            oq[0] += 1
                            oeng.dma_start(
                                out[
                                    bt * P : (bt + 1) * P,
                                    c,
                                    et * FREE : (et + 1) * FREE,
                                ],
                                dst,
                            )
                    if not last:
                        # one 1 MB DMA per (bt, c) on the SWDGE ring,
                        # keeping both HWDGE rings pure-W
                        nc.gpsimd.dma_start(out[bt * P : (bt + 1) * P, c, :], ot[:])
    nc.compile()
    return nc


def _get_nc():
    if "nc" not in _nc_cache:
        _nc_cache["nc"] = _build()
    return _nc_cache["nc"]


def _run(x, W, b, **spmd_kwargs):
    nc = _get_nc()

    in_maps = []
    for i in range(NCORES):
        c0, c1 = i * CPC, (i + 1) * CPC
        # xt[c, p, k*B + b] = x[b, k*P + p, c]
        xs = x[:, :, c0:c1].astype(np.float16)           # (B, S, CPC)
        xs = xs.transpose(2, 1, 0).reshape(CPC, KT, P, B)
        xt_i = np.ascontiguousarray(xs.transpose(0, 2, 1, 3)).reshape(
            CPC, P, KT * B
        )
        # w[c, p, k, e] = W[c, k*P + p, e]
        ws = W[c0:c1].astype(np.float16).reshape(CPC, KT, P, E)
        w_i = np.ascontiguousarray(ws.transpose(0, 2, 1, 3))
        b_i = np.ascontiguousarray(b[c0:c1].astype(np.float32))
        in_maps.append({"xt": xt_i, "w": w_i, "bias": b_i})

    res = run_bass_kernel_spmd(
        nc, in_maps, core_ids=list(range(NCORES)), **spmd_kwargs
    )
    out = np.concatenate(
        [r["out"].astype(np.float32) for r in res.results], axis=1
    )
    return out, res


def kernel(x: np.ndarray, W: np.ndarray, b: np.ndarray) -> np.ndarray:
    out, _ = _run(x, W, b)
    return out


# revision 17
# speedup vs baseline: 1.0094x; 1.0094x over previous
"""Grouped per-channel Linear + ReLU on 8 TRN2 NeuronCores.

Problem: out[b,c,e] = relu(sum_s x[b,s,c] * W[c,s,e] + bias[c,e])
  x: (256, 2048, 32) f32, W: (32, 2048, 2048) f32, bias: (32, 2048) f32
  out: (256, 32, 2048) f32

Sharding: expert/channel parallel - core i computes channels [4i, 4i+4).
Each core runs 4 independent GEMMs of (256x2048)@(2048x2048) with the
contraction dim S on SBUF partitions.

Precision: x is fp16 (stationary operand). W k-tiles 0-9 are fp16;
k-tiles 10-15 are fp8 E3M4 (the PE runs fp8e3 at full bf16 rate, and the
two operand dtypes of a matmul are independent). All W and the bias are
pre-scaled by 2^9 on the host so the fp8 values sit in E3M4's normal
range; the eviction activation unscales with scale=2^-9 (exact, power of
two). Measured rel l2 error ~8.6e-3 (fp8 on 6/16 of the contraction).

Why mixed precision: two NeuronCores share one 716 GB/s HBM stack. At
pure fp16 each core moves 42 MB -> the pair needs 117 us of DMA, just
above the 110.5 us PE floor (512 N=512 matmuls at ~216 ns), so whichever
core loses HBM arbitration stalls and the slowest core sets the score.
Dropping 6/16 of W to fp8 cuts per-core traffic to ~35.7 MB (pair ~100
us), putting the kernel safely in the PE-bound regime on every core.
The fp8 k-tiles sit at the END of each channel, halving W demand exactly
where the next channel's x-slab prefetch and the output writes burst.

Schedule: W streams in 1 MB (fp16) / 0.5 MB (fp8) 2-k-tile chunks
alternating across BOTH HWDGE rings, ~14 MB SBUF lookahead. Channel 0
ramps with single-k-tile chunks: k0-k5 as E-halves split across both
rings (need-order arrival), k6-k7 on the SWDGE ring, with ~48 zero
warmup matmuls keeping the PE busy so the HAM clock gate hits 8/8
before real work. x slabs prefetch one channel ahead on SWDGE, gated
behind a mid-channel W chunk; outputs for channels 0-2 leave on SWDGE;
the last channel's outputs leave eagerly per 512-col subtile on all
three rings. Eviction: VectorE adds the broadcast bias, ScalarE applies
ReLU + 2^-9 unscale + fp16 cast.
"""

import os
import sys

for _p in ("/opt/trn_rl_repo", "/root/.axon_site/_ro/trn_rl_repo"):
    if os.path.isdir(_p) and _p not in sys.path:
        sys.path.insert(0, _p)

import numpy as np
import ml_dtypes

import concourse.bacc as bacc
import concourse.mybir as mybir
from concourse import tile
from concourse.bass_utils import run_bass_kernel_spmd
from concourse.tile_rust import add_dep_helper

B, S, C, E = 256, 2048, 32, 2048
NCORES = 8
CPC = C // NCORES          # channels per core = 4
P = 128
KT = S // P                # 16 k-tiles
KT16 = 10                  # k-tiles 0..9 in fp16
KT8 = KT - KT16            # k-tiles 10..15 in fp8 e3m4
NBT = B // P               # 2 batch tiles
FREE = 512                 # matmul moving free dim (one PSUM bank of f32)
NET = E // FREE            # 4 e-tiles
KC = 2                     # k-tiles per W DMA chunk
WBUFS = 13                 # fp16 W chunk lookahead
W8BUFS = 6                 # fp8 W chunk lookahead (2 channels of tail)
NWARM = 48                 # PE warmup matmuls during the DMA head
WSCALE = 512.0             # host W/bias pre-scale (2^9), undone at evict

_nc_cache = {}


def _build():
    f16 = mybir.dt.float16
    f32 = mybir.dt.float32
    f8 = mybir.dt.float8e3
    nc = bacc.Bacc(None, target_bir_lowering=False)
    # xt[c, p, k*B + b] = x[b, k*P + p, c] : 8 KB contiguous per partition
    xt = nc.dram_tensor("xt", [CPC, P, KT * B], f16, kind="ExternalInput")
    # w16[c, p, k, e] = WSCALE * W[c, k*P + p, e], k < KT16
    w16 = nc.dram_tensor("w16", [CPC, P, KT16, E], f16, kind="ExternalInput")
    # w8[c, p, k, e] = WSCALE * W[c, (KT16+k)*P + p, e] in fp8 e3m4
    w8 = nc.dram_tensor("w8", [CPC, P, KT8, E], f8, kind="ExternalInput")
    bias = nc.dram_tensor("bias", [CPC, E], f32, kind="ExternalInput")
    out = nc.dram_tensor("out", [B, CPC, E], f16, kind="ExternalOutput")

    with tile.TileContext(nc) as tc:
        with (
            tc.tile_pool(name="const", bufs=1) as const,
            tc.tile_pool(name="xpool", bufs=2) as xpool,
            tc.tile_pool(name="bpool", bufs=2) as bpool,
            tc.tile_pool(name="bbpool", bufs=2) as bbpool,
            tc.tile_pool(name="ttmp", bufs=8) as ttmp,
            tc.tile_pool(name="wpool", bufs=WBUFS) as wpool,
            tc.tile_pool(name="w8pool", bufs=W8BUFS) as w8pool,
            tc.tile_pool(name="opool", bufs=4) as opool,
            tc.tile_pool(name="psum", bufs=NBT * NET, space="PSUM") as psum,
        ):
            zbias = const.tile([P, 1], f32)
            nc.any.memset(zbias[:], 0.0)
            wz = const.tile([P, P], f16)
            nc.vector.memset(wz[:], 0.0)

            xtiles: dict[int, object] = {}
            btiles: dict[int, object] = {}

            def bias_broadcast(c):
                bsb = bpool.tile([1, E], f32, name="bsb")
                nc.gpsimd.dma_start(bsb[:], bias[c : c + 1, :])
                bbc = bbpool.tile([P, E], f32, name="bbc")
                nc.gpsimd.partition_broadcast(bbc[:], bsb[:])
                btiles[c] = bbc

            # Channel 0's x: k0-1 races the first W chunk on the HWDGE
            # rings; the rest rides the otherwise-idle SWDGE ring.
            xsb0 = xpool.tile([P, KT * B], f16, name="xsb")
            nc.sync.dma_start(xsb0[:, : 2 * B], xt[0, :, : 2 * B])
            nc.gpsimd.dma_start(xsb0[:, 2 * B : 8 * B], xt[0, :, 2 * B : 8 * B])
            nc.gpsimd.dma_start(xsb0[:, 8 * B :], xt[0, :, 8 * B :])
            xtiles[0] = xsb0

            def prefetch_channel(c, after):
                # next channel's x slab + bias on the SWDGE ring, held back
                # until mid-channel so it doesn't steal HBM bandwidth from
                # the live W stream (GpSimd is in-order: gating the slab
                # gates everything behind it too)
                xsb = xpool.tile([P, KT * B], f16, name="xsb")
                xdma = nc.gpsimd.dma_start(xsb[:], xt[c, :, :])
                add_dep_helper(
                    xdma.ins,
                    after.ins,
                    reason="x prefetch waits for mid-channel W chunk",
                )
                xtiles[c] = xsb
                bias_broadcast(c)

            qtog = [0]

            def weng():
                qtog[0] ^= 1
                return nc.sync if qtog[0] else nc.scalar

            for c in range(CPC):
                xsb = xtiles[c]
                ps = [
                    [
                        psum.tile([P, FREE], f32, name="ps")
                        for _ in range(NET)
                    ]
                    for _ in range(NBT)
                ]
                if c == 0:
                    # PE warmup: ~4us of zero matmuls into the first PSUM
                    # bank while the first x/W DMAs are in flight, so the
                    # HAM clock gate reaches 8/8 before the real matmuls
                    # start. The real k0 matmul (start=True) overwrites.
                    for _ in range(NWARM):
                        nc.tensor.matmul(
                            ps[0][0][:, :P], wz[:], wz[:], start=True, stop=True
                        )

                # chunk schedule: (k-tiles, fp8?) per DMA chunk. Channel 0
                # ramps with single k-tiles; fp8 tail is always 2-k chunks.
                if c == 0:
                    chunks = [(1, False)] * 8 + [(KC, False)] + [(KC, True)] * 3
                else:
                    chunks = [(KC, False)] * 5 + [(KC, True)] * 3
                k = 0
                prefetched = False
                for ci, (ckt, is8) in enumerate(chunks):
                    if is8:
                        wsb = w8pool.tile([P, KC, E], f8, name="w8sb")
                        src = w8
                        ksrc = k - KT16
                    else:
                        wsb = wpool.tile([P, KC, E], f16, name="wsb")
                        src = w16
                        ksrc = k
                    if c == 0 and ci < 6:
                        # ramp k0-k5: E halves split across BOTH HWDGE
                        # rings (alternating) so k-tiles complete in need
                        # order even when the rings' contended rates differ
                        ea = nc.sync if ci % 2 == 0 else nc.scalar
                        eb = nc.scalar if ci % 2 == 0 else nc.sync
                        ea.dma_start(
                            wsb[:, :1, : E // 2],
                            src[c, :, ksrc : ksrc + 1, : E // 2],
                        )
                        wdma = eb.dma_start(
                            wsb[:, :1, E // 2 :],
                            src[c, :, ksrc : ksrc + 1, E // 2 :],
                        )
                    elif c == 0 and ci < 8:
                        # ramp k6-k7 ride the SWDGE ring: a third parallel
                        # delivery channel during the bandwidth-starved head
                        wdma = nc.gpsimd.dma_start(
                            wsb[:, :1, :], src[c, :, ksrc : ksrc + 1, :]
                        )
                        if ci == 7:
                            bias_broadcast(0)
                    else:
                        eng = weng()
                        wdma = eng.dma_start(
                            wsb[:, :ckt, :], src[c, :, ksrc : ksrc + ckt, :]
                        )
                    for kk in range(ckt):
                        for bt in range(NBT):
                            lhsT = xsb[:, k * B + bt * P : k * B + (bt + 1) * P]
                            for et in range(NET):
                                nc.tensor.matmul(
                                    ps[bt][et][:],
                                    lhsT,
                                    wsb[:, kk, et * FREE : (et + 1) * FREE],
                                    start=(k == 0),
                                    stop=(k == KT - 1),
                                )
                        k += 1
                    # gate at k>=10 on channel 0 so the slab queues behind
                    # the SWDGE ramp chunks rather than blocking them
                    if (
                        not prefetched
                        and k >= (10 if c == 0 else 6)
                        and c + 1 < CPC
                    ):
                        prefetch_channel(c + 1, after=wdma)
                        prefetched = True
                # Evict: VectorE adds the broadcast bias (freeing the PSUM
                # bank), ScalarE applies ReLU + 2^-9 unscale + fp16 cast.
                bbc = btiles[c]
                last = c == CPC - 1
                oq = [0]
                for bt in range(NBT):
                    ot = opool.tile([P, E], f16)
                    for et in range(NET):
                        dst = ot[:, et * FREE : (et + 1) * FREE]
                        tmp = ttmp.tile([P, FREE], f16, name="tmp")
                        nc.vector.tensor_add(
                            tmp[:],
                            ps[bt][et][:],
                            bbc[:, et * FREE : (et + 1) * FREE],
                        )
                        nc.scalar.activation(
                            dst,
                            tmp[:],
                            mybir.ActivationFunctionType.Relu,
                            bias=zbias[:],
                            scale=1.0 / WSCALE,
                        )
                        if last:
                            # tail: eager per-subtile DMAs spread over all
                            # three rings (the W stream is finished by now)
                            oengs = [
                                nc.gpsimd, nc.gpsimd, nc.sync, nc.scalar,
                                nc.sync, nc.scalar, nc.sync, nc.scalar,
                            ]
                            oeng = oengs[oq[0]]
                            oq[0] += 1
                            oeng.dma_start(
                                out[
                                    bt * P : (bt + 1) * P,
                                    c,
                                    et * FREE : (et + 1) * FREE,
                                ],
                                dst,
                            )
                    if not last:
                        # one 1 MB DMA per (bt, c) on the SWDGE ring,
                        # keeping both HWDGE rings pure-W
                        nc.gpsimd.dma_start(out[bt * P : (bt + 1) * P, c, :], ot[:])
    nc.compile()
    return nc


def _get_nc():
    if "nc" not in _nc_cache:
        _nc_cache["nc"] = _build()
    return _nc_cache["nc"]


def _run(x, W, b, **spmd_kwargs):
    nc = _get_nc()

    in_maps = []
    for i in range(NCORES):
        c0, c1 = i * CPC, (i + 1) * CPC
        # xt[c, p, k*B + b] = x[b, k*P + p, c]
        xs = x[:, :, c0:c1].astype(np.float16)           # (B, S, CPC)
        xs = xs.transpose(2, 1, 0).reshape(CPC, KT, P, B)
        xt_i = np.ascontiguousarray(xs.transpose(0, 2, 1, 3)).reshape(
            CPC, P, KT * B
        )
        # w[c, p, k, e] = WSCALE * W[c, k*P + p, e]; k<KT16 fp16, rest fp8
        ws = (W[c0:c1] * WSCALE).astype(np.float32).reshape(CPC, KT, P, E)
        w16_i = np.ascontiguousarray(
            ws[:, :KT16].transpose(0, 2, 1, 3).astype(np.float16)
        )
        w8_i = np.ascontiguousarray(
            ws[:, KT16:].transpose(0, 2, 1, 3).astype(ml_dtypes.float8_e3m4)
        )
        b_i = np.ascontiguousarray((b[c0:c1] * WSCALE).astype(np.float32))
        in_maps.append({"xt": xt_i, "w16": w16_i, "w8": w8_i, "bias": b_i})

    res = run_bass_kernel_spmd(
        nc, in_maps, core_ids=list(range(NCORES)), **spmd_kwargs
    )
    out = np.concatenate(
        [r["out"].astype(np.float32) for r in res.results], axis=1
    )
    return out, res


def kernel(x: np.ndarray, W: np.ndarray, b: np.ndarray) -> np.ndarray:
    out, _ = _run(x, W, b)
    return out


# revision 18
# speedup vs baseline: 1.0911x; 1.0810x over previous
"""Grouped per-channel Linear + ReLU on 8 TRN2 NeuronCores.

Problem: out[b,c,e] = relu(sum_s x[b,s,c] * W[c,s,e] + bias[c,e])
  x: (256, 2048, 32) f32, W: (32, 2048, 2048) f32, bias: (32, 2048) f32
  out: (256, 32, 2048) f32

Sharding: expert/channel parallel - core i computes channels [4i, 4i+4).
Each core runs 4 independent GEMMs of (256x2048)@(2048x2048) with the
contraction dim S on SBUF partitions.

Precision: x is fp16 (stationary operand). W k-tiles are fp16 or fp8
E3M4 per a schedule below (the PE runs fp8e3 at full bf16 rate, and the
two operand dtypes of a matmul are independent - verified on HW). All W
and the bias are pre-scaled by 2^9 on the host so fp8 values sit in
E3M4's normal range; the eviction activation unscales with scale=2^-9
(exact power of two). 32 of the 64 k-tiles are fp8 -> rel l2 ~1.0e-2.

Why mixed precision, and why this placement: two NeuronCores share one
716 GB/s HBM stack, and at kernel start all 8 cores pull their W
streams at once, so the first ~60 us are chip-bandwidth-starved: W
k-tiles arrive slower than the PE's ~294 GB/s consumption, the PE gaps,
and each >3.4 us gap re-throttles the HAM clock gate to 1.2 GHz. The
fp8 k-tiles are therefore concentrated in the contended window -
channel 0 (k2-15) and channel 1 (k0-9) - halving the bytes exactly
where bandwidth is scarce, plus the tails of channels 2-3 (k12-15)
which coincide with x-prefetch/output bursts. Steady state is PE-bound
at the 216 ns/matmul roofline (512 N=512 matmuls = 110.5 us/core).

Schedule: W chunks (2 k-tiles; singles for the channel-0 ramp) are
E-half split across BOTH HWDGE rings, so each half unlocks its et
matmuls independently and the PE sees work every ~0.25-0.5 MB of
delivery. Channel-0 k6-7 ride the SWDGE ring (third parallel channel
during the head); ~48 zero warmup matmuls keep the PE busy before the
first data lands so the HAM reaches 8/8 early. x slabs prefetch one
channel ahead on SWDGE, gated behind a mid-channel W chunk; outputs for
channels 0-2 leave on SWDGE; the last channel's leave eagerly per
512-col subtile on all three rings. Eviction: VectorE adds the
broadcast bias, ScalarE applies ReLU + 2^-9 unscale + fp16 cast.
"""

import os
import sys

for _p in ("/opt/trn_rl_repo", "/root/.axon_site/_ro/trn_rl_repo"):
    if os.path.isdir(_p) and _p not in sys.path:
        sys.path.insert(0, _p)

import numpy as np
import ml_dtypes

import concourse.bacc as bacc
import concourse.mybir as mybir
from concourse import tile
from concourse.bass_utils import run_bass_kernel_spmd
from concourse.tile_rust import add_dep_helper

B, S, C, E = 256, 2048, 32, 2048
NCORES = 8
CPC = C // NCORES          # channels per core = 4
P = 128
KT = S // P                # 16 k-tiles
NBT = B // P               # 2 batch tiles
FREE = 512                 # matmul moving free dim (one PSUM bank of f32)
NET = E // FREE            # 4 e-tiles
KC = 2                     # k-tiles per W DMA chunk
WBUFS = 10                 # fp16 W chunk lookahead
W8BUFS = 12                # fp8 W chunk lookahead
NWARM = 48                 # PE warmup matmuls during the DMA head
WSCALE = 512.0             # host W/bias pre-scale (2^9), undone at evict

# per-channel fp8 k-tiles: concentrated in the bandwidth-starved first
# ~60us (all of ch0 after the first two tiles, first 10 tiles of ch1)
# plus the channel tails that coincide with prefetch/output bursts.
K8 = {
    0: frozenset(range(2, 16)),
    1: frozenset(range(0, 10)),
    2: frozenset({12, 13, 14, 15}),
    3: frozenset({12, 13, 14, 15}),
}

_nc_cache = {}


def _chunks(c):
    """[(k0, nkt, is8)] covering k 0..15; singles below k8 on channel 0."""
    out = []
    k = 0
    while k < KT:
        is8 = k in K8[c]
        n = 1
        if (
            not (c == 0 and k < 8)
            and k + 1 < KT
            and ((k + 1) in K8[c]) == is8
        ):
            n = KC
        out.append((k, n, is8))
        k += n
    return out


def _build():
    f16 = mybir.dt.float16
    f32 = mybir.dt.float32
    f8 = mybir.dt.float8e3
    nc = bacc.Bacc(None, target_bir_lowering=False)
    # xt[c, p, k*B + b] = x[b, k*P + p, c] : 8 KB contiguous per partition
    xt = nc.dram_tensor("xt", [CPC, P, KT * B], f16, kind="ExternalInput")
    # w16[c, p, j, e] / w8[c, p, j, e]: j-th fp16/fp8 k-tile of channel c
    # (k-tiles packed per dtype in k order), value WSCALE * W[c, k*P+p, e]
    n16 = sum(KT - len(K8[c]) for c in range(CPC))
    n8 = sum(len(K8[c]) for c in range(CPC))
    w16 = nc.dram_tensor("w16", [n16, P, E], f16, kind="ExternalInput")
    w8 = nc.dram_tensor("w8", [n8, P, E], f8, kind="ExternalInput")
    bias = nc.dram_tensor("bias", [CPC, E], f32, kind="ExternalInput")
    out = nc.dram_tensor("out", [B, CPC, E], f16, kind="ExternalOutput")

    # DRAM slot index of (c, k) within its dtype-packed tensor
    slot16, slot8 = {}, {}
    i16 = i8 = 0
    for c in range(CPC):
        for k in range(KT):
            if k in K8[c]:
                slot8[(c, k)] = i8
                i8 += 1
            else:
                slot16[(c, k)] = i16
                i16 += 1

    with tile.TileContext(nc) as tc:
        with (
            tc.tile_pool(name="const", bufs=1) as const,
            tc.tile_pool(name="xpool", bufs=2) as xpool,
            tc.tile_pool(name="bpool", bufs=1) as bpool,
            tc.tile_pool(name="bbpool", bufs=2) as bbpool,
            tc.tile_pool(name="ttmp", bufs=8) as ttmp,
            tc.tile_pool(name="wpool", bufs=WBUFS) as wpool,
            tc.tile_pool(name="w8pool", bufs=W8BUFS) as w8pool,
            tc.tile_pool(name="opool", bufs=4) as opool,
            tc.tile_pool(name="psum", bufs=NBT * NET, space="PSUM") as psum,
        ):
            zbias = const.tile([P, 1], f32)
            nc.any.memset(zbias[:], 0.0)
            wz = const.tile([P, P], f16)
            nc.vector.memset(wz[:], 0.0)

            xtiles: dict[int, object] = {}
            btiles: dict[int, object] = {}

            def bias_broadcast(c):
                bsb = bpool.tile([1, E], f32, name="bsb")
                nc.gpsimd.dma_start(bsb[:], bias[c : c + 1, :])
                bbc = bbpool.tile([P, E], f32, name="bbc")
                nc.gpsimd.partition_broadcast(bbc[:], bsb[:])
                btiles[c] = bbc

            # Channel 0's x: k0-1 races the first W chunk on the HWDGE
            # rings; the rest rides the otherwise-idle SWDGE ring.
            xsb0 = xpool.tile([P, KT * B], f16, name="xsb")
            nc.sync.dma_start(xsb0[:, : 2 * B], xt[0, :, : 2 * B])
            nc.gpsimd.dma_start(xsb0[:, 2 * B : 8 * B], xt[0, :, 2 * B : 8 * B])
            nc.gpsimd.dma_start(xsb0[:, 8 * B :], xt[0, :, 8 * B :])
            xtiles[0] = xsb0

            def prefetch_channel(c, after):
                # next channel's x slab + bias on the SWDGE ring, held back
                # until mid-channel so it doesn't steal HBM bandwidth from
                # the live W stream (GpSimd is in-order: gating the slab
                # gates everything behind it too)
                xsb = xpool.tile([P, KT * B], f16, name="xsb")
                xdma = nc.gpsimd.dma_start(xsb[:], xt[c, :, :])
                add_dep_helper(
                    xdma.ins,
                    after.ins,
                    reason="x prefetch waits for mid-channel W chunk",
                )
                xtiles[c] = xsb
                bias_broadcast(c)

            htog = [0]
            for c in range(CPC):
                xsb = xtiles[c]
                ps = [
                    [
                        psum.tile([P, FREE], f32, name="ps")
                        for _ in range(NET)
                    ]
                    for _ in range(NBT)
                ]
                if c == 0:
                    # PE warmup: ~4us of zero matmuls into the first PSUM
                    # bank while the first x/W DMAs are in flight, so the
                    # HAM clock gate reaches 8/8 before the real matmuls
                    # start. The real k0 matmul (start=True) overwrites.
                    for _ in range(NWARM):
                        nc.tensor.matmul(
                            ps[0][0][:, :P], wz[:], wz[:], start=True, stop=True
                        )

                prefetched = False
                for ci, (k0, nkt, is8) in enumerate(_chunks(c)):
                    if is8:
                        wsb = w8pool.tile([P, KC, E], f8, name="w8sb")
                        src, slot = w8, slot8[(c, k0)]
                    else:
                        wsb = wpool.tile([P, KC, E], f16, name="wsb")
                        src, slot = w16, slot16[(c, k0)]

                    def span(h0, h1):
                        # DRAM view of k-tiles slot..slot+nkt-1, cols h0:h1
                        return src[slot : slot + nkt, :, h0:h1].rearrange(
                            "k p e -> p k e"
                        )

                    if c == 0 and 6 <= k0 < 8:
                        # ramp k6-k7 ride the SWDGE ring: a third parallel
                        # delivery channel during the bandwidth-starved head
                        wdma = nc.gpsimd.dma_start(wsb[:, :nkt, :], span(0, E))
                        halves = [(0, NET)]
                        if k0 == 7:
                            bias_broadcast(0)
                    else:
                        # E-half split across BOTH HWDGE rings: each half
                        # unlocks its et matmuls as soon as it lands, and
                        # the per-ring FIFO keeps k-tiles in need order
                        htog[0] ^= 1
                        ea = nc.sync if htog[0] else nc.scalar
                        eb = nc.scalar if htog[0] else nc.sync
                        ea.dma_start(wsb[:, :nkt, : E // 2], span(0, E // 2))
                        wdma = eb.dma_start(
                            wsb[:, :nkt, E // 2 :], span(E // 2, E)
                        )
                        halves = [(0, NET // 2), (NET // 2, NET)]

                    for e0, e1 in halves:
                        for kk in range(nkt):
                            k = k0 + kk
                            for bt in range(NBT):
                                lhsT = xsb[
                                    :, k * B + bt * P : k * B + (bt + 1) * P
                                ]
                                for et in range(e0, e1):
                                    nc.tensor.matmul(
                                        ps[bt][et][:],
                                        lhsT,
                                        wsb[:, kk, et * FREE : (et + 1) * FREE],
                                        start=(k == 0),
                                        stop=(k == KT - 1),
                                    )
                    # gate the next channel's x prefetch behind a
                    # mid-channel W chunk (later on ch0: behind the SWDGE
                    # ramp so the slab doesn't block it)
                    if (
                        not prefetched
                        and k0 + nkt >= (10 if c == 0 else 6)
                        and c + 1 < CPC
                    ):
                        prefetch_channel(c + 1, after=wdma)
                        prefetched = True
                # Evict: VectorE adds the broadcast bias (freeing the PSUM
                # bank), ScalarE applies ReLU + 2^-9 unscale + fp16 cast.
                bbc = btiles[c]
                last = c == CPC - 1
                oq = [0]
                for bt in range(NBT):
                    ot = opool.tile([P, E], f16)
                    for et in range(NET):
                        dst = ot[:, et * FREE : (et + 1) * FREE]
                        tmp = ttmp.tile([P, FREE], f16, name="tmp")
                        nc.vector.tensor_add(
                            tmp[:],
                            ps[bt][et][:],
                            bbc[:, et * FREE : (et + 1) * FREE],
                        )
                        nc.scalar.activation(
                            dst,
                            tmp[:],
                            mybir.ActivationFunctionType.Relu,
                            bias=zbias[:],
                            scale=1.0 / WSCALE,
                        )
                        if last:
                            # tail: eager per-subtile DMAs spread over all
                            # three rings (the W stream is finished by now)
                            oengs = [
                                nc.gpsimd, nc.gpsimd, nc.sync, nc.scalar,
                                nc.sync, nc.scalar, nc.sync, nc.scalar,
                            ]
                            oeng = oengs[oq[0]]
                            oq[0] += 1
                            oeng.dma_start(
                                out[
                                    bt * P : (bt + 1) * P,
                                    c,
                                    et * FREE : (et + 1) * FREE,
                                ],
                                dst,
                            )
                    if not last:
                        # one 1 MB DMA per (bt, c) on the SWDGE ring,
                        # keeping both HWDGE rings pure-W
                        nc.gpsimd.dma_start(out[bt * P : (bt + 1) * P, c, :], ot[:])
    nc.compile()
    return nc


def _get_nc():
    if "nc" not in _nc_cache:
        _nc_cache["nc"] = _build()
    return _nc_cache["nc"]


def _run(x, W, b, **spmd_kwargs):
    nc = _get_nc()

    in_maps = []
    for i in range(NCORES):
        c0, c1 = i * CPC, (i + 1) * CPC
        # xt[c, p, k*B + b] = x[b, k*P + p, c]
        xs = x[:, :, c0:c1].astype(np.float16)           # (B, S, CPC)
        xs = xs.transpose(2, 1, 0).reshape(CPC, KT, P, B)
        xt_i = np.ascontiguousarray(xs.transpose(0, 2, 1, 3)).reshape(
            CPC, P, KT * B
        )
        # dtype-packed k-tiles, value WSCALE * W[c, k*P + p, e]
        ws = (W[c0:c1] * WSCALE).astype(np.float32).reshape(CPC, KT, P, E)
        t16 = [ws[c, k] for c in range(CPC) for k in range(KT)
               if k not in K8[c]]
        t8 = [ws[c, k] for c in range(CPC) for k in range(KT) if k in K8[c]]
        w16_i = np.ascontiguousarray(np.stack(t16).astype(np.float16))
        w8_i = np.ascontiguousarray(
            np.stack(t8).astype(ml_dtypes.float8_e3m4)
        )
        b_i = np.ascontiguousarray((b[c0:c1] * WSCALE).astype(np.float32))
        in_maps.append({"xt": xt_i, "w16": w16_i, "w8": w8_i, "bias": b_i})

    res = run_bass_kernel_spmd(
        nc, in_maps, core_ids=list(range(NCORES)), **spmd_kwargs
    )
    out = np.concatenate(
        [r["out"].astype(np.float32) for r in res.results], axis=1
    )
    return out, res


def kernel(x: np.ndarray, W: np.ndarray, b: np.ndarray) -> np.ndarray:
    out, _ = _run(x, W, b)
    return out
